# revision 4
# baseline (speedup 1.0000x reference)
"""CapsuleMaxPooling Trainium2 kernel.

Problem: inp [B=32, C=32, H=64, W=64, D=8] f32, kernel_size k=2.
For each 2x2 spatial window pick the capsule vector (length D=8) with the
largest squared L2 norm (first-max tie-break) -> out [B, C, 32, 32, 8].

Strategy (fully data-parallel, shard B across 8 cores; per core the shard is
viewed as rows r=(b, c, hk) of 1024 contiguous floats = (dh, wk, dw, d);
32 row-tiles of 128 partitions, processed as groups of up to 8 row-tiles).

The 20 MiB/core of HBM traffic bounds the kernel at ~58us; engines:
  - ACT: sq = x^2 per row-tile + one base copy of candidate D per group.
  - DVE: tree level-1 add of sq d-halves (8->4); 3-op tournament per group
    (pairwise max, final max, then ONE is_ge producing all three masks
    [pos=3, wk] against a stride-0-broadcast M); 3 copy_predicated per
    group. copy_predicated needs an integer mask: int32 bitcast of the f32
    0.0/1.0 mask broadcast over d via a stride-0 inner dim.
  - GPSIMD (Pool): tree levels 2+3 (4->2->1, add only); level 3 writes
    norms transposed to [pos, wk] so the tournament reads contiguously.
  - Predication ORDER (D base, then C, then B, then A last) gives exact
    first-argmax semantics.
  - HWDGE (nc.sync) DMAs, partition-major contiguous chunks per tile.
  - Groups are software-pipelined by one: group g-1's tournament+copies
    issue after group g's loads so the DVE never idles waiting on masks.
"""

import numpy as np

try:
    import concourse.bass as bass
except ImportError:  # pragma: no cover
    import sys

    sys.path.insert(0, "/opt/trn_rl_repo")
    import concourse.bass as bass

from concourse import bacc, mybir
from concourse.bass_utils import run_bass_kernel_spmd
from concourse.tile import TileContext

P = 128
N_CORES = 8
ROW_W = 1024  # (dh=2) * (wk=32) * (dw=2) * (d=8)
OUT_W = 256  # (wk=32) * (d=8)
DEFAULT_SCHED = ((1, 1), (2, 4), (4, 4), (4, 4), (4, 2), (1, 1))


def _b0(a, n, pos):
    """Insert a stride-0 dim of extent n at free-dim position pos."""
    ap = list(a.ap)
    ap.insert(pos, [0, n])
    return bass.AP(tensor=a.tensor, offset=a.offset, ap=ap)


def build_nc(R=4096, sched=DEFAULT_SCHED):
    """Build the per-core Bass program. R = rows (b,c,hk) per core."""
    f32 = mybir.dt.float32
    i32 = mybir.dt.int32
    add = mybir.AluOpType.add
    mx = mybir.AluOpType.max
    nc = bacc.Bacc(None, target_bir_lowering=False)
    x = nc.dram_tensor("x", [R, ROW_W], f32, kind="ExternalInput")
    y = nc.dram_tensor("y", [R, OUT_W], f32, kind="ExternalOutput")
    assert sum(sum(g) for g in sched) * P == R

    with TileContext(nc) as tc:
        with (
            tc.tile_pool(name="xp", bufs=3) as xp,
            tc.tile_pool(name="sqp", bufs=2) as sqp,
            tc.tile_pool(name="s4p", bufs=2) as s4p,
            tc.tile_pool(name="s2p", bufs=2) as s2p,
            tc.tile_pool(name="normp", bufs=2) as normp,
            tc.tile_pool(name="maskp", bufs=2) as maskp,
            tc.tile_pool(name="outp", bufs=3) as outp,
        ):

            def load_group(grp, tile0):
                gtb = sum(grp)
                xg = xp.tile([P, gtb, ROW_W], f32, tag="xg")
                nt = normp.tile([P, gtb, 4, 32], f32, tag="nt")
                t, q0 = tile0, 0
                for tb in grp:
                    r0 = t * P
                    xs = xg[:, q0 : q0 + tb]
                    nc.sync.dma_start(
                        out=xs,
                        in_=x[r0 : r0 + tb * P, :].rearrange(
                            "(p j) c -> p j c", p=P
                        ),
                    )
                    sq = sqp.tile([P, tb, ROW_W], f32, tag="sq")
                    nc.scalar.square(sq, xs)
                    # level 1 on DVE: d 8 -> 4
                    sqv = sq.rearrange("p j (g d) -> p j g d", d=8)
                    s4 = s4p.tile([P, tb, 128, 4], f32, tag="s4")
                    nc.vector.tensor_tensor(
                        s4, sqv[:, :, :, 0:4], sqv[:, :, :, 4:8], op=add
                    )
                    # levels 2+3 on gpsimd: 4 -> 2 -> 1 (transposed out)
                    s2 = s2p.tile([P, tb, 128, 2], f32, tag="s2")
                    nc.gpsimd.tensor_tensor(
                        s2, s4[:, :, :, 0:2], s4[:, :, :, 2:4], op=add
                    )
                    s2v = s2.rearrange(
                        "p j (dh wk dw) e -> p j dh wk dw e", dh=2, wk=32
                    )
                    ntv = nt[:, q0 : q0 + tb].rearrange(
                        "p j (dh dw) wk -> p j dh wk dw", dh=2
                    )
                    nc.gpsimd.tensor_tensor(
                        ntv, s2v[:, :, :, :, :, 0], s2v[:, :, :, :, :, 1],
                        op=add,
                    )
                    q0 += tb
                    t += tb
                # base candidate D for the whole group (ACT)
                ot = outp.tile([P, gtb, 32, 8], f32, tag="ot")
                xr = xg.rearrange(
                    "p j (dh wk dw d) -> p j dh wk dw d", dh=2, dw=2, d=8
                )
                nc.scalar.copy(ot, xr[:, :, 1, :, 1, :])
                return dict(grp=grp, gtb=gtb, nt=nt, xg=xg, ot=ot, tile0=tile0)

            def select_group(st):
                grp, gtb, nt = st["grp"], st["gtb"], st["nt"]
                xg, ot = st["xg"], st["ot"]
                # 3-op tournament: h12 = pairwise max, M = final max,
                # wABC = (n[pos] >= M) for pos in {A,B,C} in one op.
                h12 = maskp.tile([P, gtb, 2, 32], f32, tag="h12")
                nc.vector.tensor_tensor(
                    h12, nt[:, :, 0:2, :], nt[:, :, 2:4, :], op=mx
                )
                M = maskp.tile([P, gtb, 32], f32, tag="M")
                nc.vector.tensor_tensor(
                    M, h12[:, :, 0, :], h12[:, :, 1, :], op=mx
                )
                wABC = maskp.tile([P, gtb, 3, 32], f32, tag="wABC")
                nc.vector.tensor_tensor(
                    wABC, nt[:, :, 0:3, :], _b0(M[:, :, :], 3, 2),
                    op=mybir.AluOpType.is_ge,
                )
                xr = xg.rearrange(
                    "p j (dh wk dw d) -> p j dh wk dw d", dh=2, dw=2, d=8
                )
                # overwrite order C, B, A => first-argmax on ties
                for pos, dh, dw in ((2, 1, 0), (1, 0, 1), (0, 0, 0)):
                    m = wABC[:, :, pos, :].bitcast(i32)
                    nc.vector.copy_predicated(
                        ot, _b0(m, 8, 3), xr[:, :, dh, :, dw, :]
                    )
                t, q0 = st["tile0"], 0
                for tb in grp:
                    r0 = t * P
                    nc.sync.dma_start(
                        out=y[r0 : r0 + tb * P, :].rearrange(
                            "(p j) c -> p j c", p=P
                        ),
                        in_=ot[:, q0 : q0 + tb].rearrange(
                            "p j w d -> p j (w d)"
                        ),
                    )
                    q0 += tb
                    t += tb

            pend = None
            tile0 = 0
            for grp in sched:
                st = load_group(list(grp), tile0)
                tile0 += sum(grp)
                if pend is not None:
                    select_group(pend)
                pend = st
            select_group(pend)
    nc.compile()
    return nc


_NC_CACHE = {}


def _get_nc(R):
    if R not in _NC_CACHE:
        _NC_CACHE[R] = build_nc(R)
    return _NC_CACHE[R]


def kernel(inp, kernel_size):
    inp = np.asarray(inp)
    k = int(np.asarray(kernel_size))
    assert k == 2, f"kernel hardcoded for kernel_size=2, got {k}"
    B, C, H, W, D = inp.shape
    assert (B, C, H, W, D) == (32, 32, 64, 64, 8), inp.shape
    Hk, Wk = H // k, W // k

    bs = B // N_CORES  # 4 batches per core
    R = bs * C * Hk  # 4096 rows per core
    nc = _get_nc(R)

    in_maps = []
    for c in range(N_CORES):
        shard = np.ascontiguousarray(inp[c * bs : (c + 1) * bs]).reshape(R, ROW_W)
        in_maps.append({"x": shard})

    res = run_bass_kernel_spmd(nc, in_maps, list(range(N_CORES)))
    out = np.concatenate(
        [r["y"].reshape(bs, C, Hk, Wk, D) for r in res.results], axis=0
    )
    return out


# revision 5
# speedup vs baseline: 1.0319x; 1.0319x over previous
"""CapsuleMaxPooling Trainium2 kernel.

Problem: inp [B=32, C=32, H=64, W=64, D=8] f32, kernel_size k=2.
For each 2x2 spatial window pick the capsule vector (length D=8) with the
largest squared L2 norm (first-max tie-break) -> out [B, C, 32, 32, 8].

Strategy (fully data-parallel, shard B across 8 cores; per core the shard is
viewed as rows r=(b, c, hk) of 1024 contiguous floats = (dh, wk, dw, d);
32 row-tiles of 128 partitions, processed in groups of up to 6 row-tiles,
each group DMA'd in two half-group chunks).

The 20 MiB/core of HBM traffic bounds the kernel at ~58us; engines:
  - ACT: sq = x^2 per half-group + one base copy of candidate D per group.
  - DVE: tree level-1 add of sq d-halves (8->4); level-3 add (2->1,
    written transposed to [pos, wk]); 3-op tournament per group (pairwise
    max, final max, one is_ge producing all three masks against a
    stride-0-broadcast M); 3 copy_predicated per group (int32-bitcast
    f32 mask broadcast over d via a stride-0 inner dim).
  - GPSIMD (Pool): ONLY tree level-2 (4->2, add) - one stage, issued
    early, so DVE never waits on a fresh gpsimd result and the shared
    SBUF port sees minimal contention.
  - Predication ORDER (D base, then C, then B, then A last) gives exact
    first-argmax semantics.
  - HWDGE (nc.sync) DMAs, partition-major contiguous chunks.
  - Groups are software-pipelined by one (select of group g-1 issues
    after the loads of group g); xg has 4 buffers so the input DMA keeps
    streaming ~2 groups ahead of the selection.
"""

import numpy as np

try:
    import concourse.bass as bass
except ImportError:  # pragma: no cover
    import sys

    sys.path.insert(0, "/opt/trn_rl_repo")
    import concourse.bass as bass

from concourse import bacc, mybir
from concourse.bass_utils import run_bass_kernel_spmd
from concourse.tile import TileContext

P = 128
N_CORES = 8
ROW_W = 1024  # (dh=2) * (wk=32) * (dw=2) * (d=8)
OUT_W = 256  # (wk=32) * (d=8)
DEFAULT_SCHED = ((1, 1), (3, 3), (3, 3), (3, 3), (3, 3), (2, 2), (1, 1))


def _b0(a, n, pos):
    """Insert a stride-0 dim of extent n at free-dim position pos."""
    ap = list(a.ap)
    ap.insert(pos, [0, n])
    return bass.AP(tensor=a.tensor, offset=a.offset, ap=ap)


def build_nc(R=4096, sched=DEFAULT_SCHED):
    """Build the per-core Bass program. R = rows (b,c,hk) per core."""
    f32 = mybir.dt.float32
    i32 = mybir.dt.int32
    add = mybir.AluOpType.add
    mx = mybir.AluOpType.max
    nc = bacc.Bacc(None, target_bir_lowering=False)
    x = nc.dram_tensor("x", [R, ROW_W], f32, kind="ExternalInput")
    y = nc.dram_tensor("y", [R, OUT_W], f32, kind="ExternalOutput")
    assert sum(sum(g) for g in sched) * P == R

    with TileContext(nc) as tc:
        with (
            tc.tile_pool(name="xp", bufs=4) as xp,
            tc.tile_pool(name="sqp", bufs=2) as sqp,
            tc.tile_pool(name="s4p", bufs=3) as s4p,
            tc.tile_pool(name="s2p", bufs=3) as s2p,
            tc.tile_pool(name="normp", bufs=2) as normp,
            tc.tile_pool(name="maskp", bufs=2) as maskp,
            tc.tile_pool(name="outp", bufs=3) as outp,
        ):

            def load_group(grp, tile0):
                """DMA-in, squares, tree L1 (DVE) + L2 (gpsimd), base copy."""
                gtb = sum(grp)
                xg = xp.tile([P, gtb, ROW_W], f32, tag="xg")
                s2s, qoffs = [], []
                t, q0 = tile0, 0
                for tb in grp:
                    r0 = t * P
                    xs = xg[:, q0 : q0 + tb]
                    nc.sync.dma_start(
                        out=xs,
                        in_=x[r0 : r0 + tb * P, :].rearrange(
                            "(p j) c -> p j c", p=P
                        ),
                    )
                    sq = sqp.tile([P, tb, ROW_W], f32, tag="sq")
                    nc.scalar.square(sq, xs)
                    sqv = sq.rearrange("p j (g d) -> p j g d", d=8)
                    s4 = s4p.tile([P, tb, 128, 4], f32, tag="s4")
                    nc.vector.tensor_tensor(
                        s4, sqv[:, :, :, 0:4], sqv[:, :, :, 4:8], op=add
                    )
                    s2 = s2p.tile([P, tb, 128, 2], f32, tag="s2")
                    nc.gpsimd.tensor_tensor(
                        s2, s4[:, :, :, 0:2], s4[:, :, :, 2:4], op=add
                    )
                    s2s.append(s2)
                    qoffs.append(q0)
                    q0 += tb
                    t += tb
                ot = outp.tile([P, gtb, 32, 8], f32, tag="ot")
                xr = xg.rearrange(
                    "p j (dh wk dw d) -> p j dh wk dw d", dh=2, dw=2, d=8
                )
                nc.scalar.copy(ot, xr[:, :, 1, :, 1, :])
                return dict(
                    grp=grp, gtb=gtb, xg=xg, ot=ot, s2s=s2s, qoffs=qoffs,
                    tile0=tile0,
                )

            def select_group(st):
                """L3 (DVE, transposed), tournament, copies, DMA-out."""
                grp, gtb = st["grp"], st["gtb"]
                xg, ot = st["xg"], st["ot"]
                nt = normp.tile([P, gtb, 4, 32], f32, tag="nt")
                for tb, q0, s2 in zip(grp, st["qoffs"], st["s2s"]):
                    s2v = s2.rearrange(
                        "p j (dh wk dw) e -> p j dh wk dw e", dh=2, wk=32
                    )
                    ntv = nt[:, q0 : q0 + tb].rearrange(
                        "p j (dh dw) wk -> p j dh wk dw", dh=2
                    )
                    nc.vector.tensor_tensor(
                        ntv, s2v[:, :, :, :, :, 0], s2v[:, :, :, :, :, 1],
                        op=add,
                    )
                h12 = maskp.tile([P, gtb, 2, 32], f32, tag="h12")
                nc.vector.tensor_tensor(
                    h12, nt[:, :, 0:2, :], nt[:, :, 2:4, :], op=mx
                )
                M = maskp.tile([P, gtb, 32], f32, tag="M")
                nc.vector.tensor_tensor(
                    M, h12[:, :, 0, :], h12[:, :, 1, :], op=mx
                )
                wABC = maskp.tile([P, gtb, 3, 32], f32, tag="wABC")
                nc.vector.tensor_tensor(
                    wABC, nt[:, :, 0:3, :], _b0(M[:, :, :], 3, 2),
                    op=mybir.AluOpType.is_ge,
                )
                xr = xg.rearrange(
                    "p j (dh wk dw d) -> p j dh wk dw d", dh=2, dw=2, d=8
                )
                # overwrite order C, B, A => first-argmax on ties
                for pos, dh, dw in ((2, 1, 0), (1, 0, 1), (0, 0, 0)):
                    m = wABC[:, :, pos, :].bitcast(i32)
                    nc.vector.copy_predicated(
                        ot, _b0(m, 8, 3), xr[:, :, dh, :, dw, :]
                    )
                t, q0 = st["tile0"], 0
                for tb in grp:
                    r0 = t * P
                    nc.sync.dma_start(
                        out=y[r0 : r0 + tb * P, :].rearrange(
                            "(p j) c -> p j c", p=P
                        ),
                        in_=ot[:, q0 : q0 + tb].rearrange(
                            "p j w d -> p j (w d)"
                        ),
                    )
                    q0 += tb
                    t += tb

            pend = None
            tile0 = 0
            for grp in sched:
                st = load_group(list(grp), tile0)
                tile0 += sum(grp)
                if pend is not None:
                    select_group(pend)
                pend = st
            select_group(pend)
    nc.compile()
    return nc


_NC_CACHE = {}


def _get_nc(R):
    if R not in _NC_CACHE:
        _NC_CACHE[R] = build_nc(R)
    return _NC_CACHE[R]


def kernel(inp, kernel_size):
    inp = np.asarray(inp)
    k = int(np.asarray(kernel_size))
    assert k == 2, f"kernel hardcoded for kernel_size=2, got {k}"
    B, C, H, W, D = inp.shape
    assert (B, C, H, W, D) == (32, 32, 64, 64, 8), inp.shape
    Hk, Wk = H // k, W // k

    bs = B // N_CORES  # 4 batches per core
    R = bs * C * Hk  # 4096 rows per core
    nc = _get_nc(R)

    in_maps = []
    for c in range(N_CORES):
        shard = np.ascontiguousarray(inp[c * bs : (c + 1) * bs]).reshape(R, ROW_W)
        in_maps.append({"x": shard})

    res = run_bass_kernel_spmd(nc, in_maps, list(range(N_CORES)))
    out = np.concatenate(
        [r["y"].reshape(bs, C, Hk, Wk, D) for r in res.results], axis=0
    )
    return out
